# revision 26
# baseline (speedup 1.0000x reference)
"""Trainium2 Bass kernel for nn_CapXLayer (CapsNet-style layer).

Sharding: data-parallel over batch. 8 batches -> 8 NeuronCores, one batch
per core. All parameters replicated. Full inputs in, full output out.

Per-core dataflow v2 (CH layout [channels, pixels], px chunks of 512,
routing processed in chunk PAIRS so all [64,*]-shaped elementwise work runs
on full-width [128,512] tiles):

  conv: relu(x) -> conv1 (1x1 grouped) writes a persistent flat h1 [128,64,64]
        per half; conv2 (3x3 SAME) accumulates 9 taps in PSUM with
        row/column-trimmed access patterns (no zero-padded staging, edge
        columns simply skip the out-of-range taps); conv3 per capsule.
  routing: u16[ic] bf16 tiles; nsq = PE-reduce of u16^2 (DVE bf16 squares);
        squash factors + sigmoids computed once per PAIR on [128,512] tiles
        (chunk A in partitions 0:64, chunk B in 64:128); s/d mixes via
        per-ic PE replicate + TT + PE accumulate, TTs split DVE/Pool.
  tail: spatial attention stats entirely via PE reductions (no DMA
        round-trips); mh accumulated for free via ACT accum_out on the sf
        copies; final residual mix split across DVE/Pool.
"""

import numpy as np

import concourse.bass as bass
import concourse.bacc as bacc
import concourse.tile as tile
import concourse.mybir as mybir
from concourse.bass_utils import run_bass_kernel_spmd

F32 = mybir.dt.float32
F32R = mybir.dt.float32r
BF16 = mybir.dt.bfloat16
AF = mybir.ActivationFunctionType
OP = mybir.AluOpType

IC, IND, MID, OC, OD = 8, 16, 32, 8, 16
B, H, W = 8, 64, 64
PX = H * W            # 4096
CS = 512              # pixels per chunk
NCH = PX // CS        # 8 chunks
RPC = CS // W         # rows per chunk = 8

# tap order: (0,0) first so the start=True matmul covers the full tile
TAPS = [(0, 0)] + [(dy, dx) for dy in (-1, 0, 1) for dx in (-1, 0, 1)
                   if (dy, dx) != (0, 0)]


# ---------------------------------------------------------------- host prep
def _prep_consts(w1, b1, w2, b2, w3, b3, attn_w, attn_b):
    """Precompute matmul-ready weight layouts and constant matrices."""
    import ml_dtypes
    c = {}
    # conv1 lhsT: [128, 128]; rows 64h..64h+63 hold half h's lhsT so the
    # lhsT slice shares its base partition with the rhs x-slice
    w1L = np.zeros((128, 128), np.float32)
    for h in range(2):
        for g in range(4):
            gg = h * 4 + g
            w1L[64 * h + g * 16:64 * h + (g + 1) * 16,
                g * 32:(g + 1) * 32] = w1[gg * 32:(gg + 1) * 32, :, 0, 0].T
    c["w1L"] = w1L
    # conv2 lhsT: [128, 2, 9, 128] in TAPS order
    w2L = np.zeros((128, 2, 9, 128), np.float32)
    for h in range(2):
        for t, (dy, dx) in enumerate(TAPS):
            for g in range(4):
                gg = h * 4 + g
                w2L[g * 32:(g + 1) * 32, h, t, g * 32:(g + 1) * 32] = \
                    w2[gg * 32:(gg + 1) * 32, :, dy + 1, dx + 1].T
    c["w2L"] = w2L
    # conv3 lhsT: [96, 8, 128].  ic's lhsT sits at rows 32g..32g+31 where
    # g = ic%4 for g<3 (partition-aligned with its h2 slice); the g==3
    # capsules are staged to base 0 (PE quadrant 3 is unusable), so their
    # lhsT sits at rows 0..31.
    w3L = np.zeros((96, 8, 128), np.float32)
    for ic in range(IC):
        g = ic % 4
        r0 = 32 * g if g < 3 else 0
        w3L[r0:r0 + 32, ic, :] = w3[ic * 128:(ic + 1) * 128, :, 0, 0].T
    c["w3L"] = w3L
    # biases as per-partition columns
    c["b1s"] = np.stack([b1[0:128], b1[128:256]], axis=1).astype(np.float32)
    c["b2s"] = np.stack([b2[0:128], b2[128:256]], axis=1).astype(np.float32)
    c["b3s"] = b3.reshape(IC, 128).T.astype(np.float32).copy()      # [128, 8]
    p = np.arange(128)
    # redM16[:, ic, :]: [128 (oc,od), 64 (ic',oc')] masked ones reduce —
    # sums od into rows 8ic..8ic+7 only.
    redM = np.zeros((128, 8, 64), np.float32)
    for ic in range(IC):
        for ocv in range(OC):
            redM[ocv * OD:(ocv + 1) * OD, ic, ic * 8 + ocv] = 1.0
    c["redM16"] = redM.astype(ml_dtypes.bfloat16)
    # onesB16: [128 (oc,od), 64 (ic',oc')] replicate-over-ic reduce of od
    j = np.arange(64)
    c["onesB16"] = (p[:, None] // OD == (j % OC)[None, :]).astype(
        ml_dtypes.bfloat16)
    # repM_both[:, ic, :]: [128, 8, 128] — rows 0:64 and 64:128 both hold
    # the [64,(ic),128] od-replication mask so either half of a pair tile
    # can be the matmul operand base.
    repM = np.zeros((64, 8, 128), np.float32)
    for ic in range(IC):
        for ocv in range(OC):
            repM[ic * 8 + ocv, ic, ocv * OD:(ocv + 1) * OD] = 1.0
    repM_both = np.zeros((128, 8, 128), np.float32)
    repM_both[0:64] = repM
    repM_both[64:128] = repM
    c["repM_both"] = repM_both
    c["i128_16"] = np.eye(128, dtype=ml_dtypes.bfloat16)
    # 0.5-scaled identity: folds iter-1's uniform c=0.5 into the accumulate
    # matmul so the TT products stay exactly usq = g*u (reused in iter 2)
    c["ihalf_16"] = (0.5 * np.eye(128)).astype(ml_dtypes.bfloat16)
    # ---- tail constants ----
    # onesA_all[:, c, :]: [128, 8, 64] — sums od of chunk c's tile into
    # rows 8c..8c+7 of the accumulated avg64 psum
    onesA_all = np.zeros((128, 8, 64), np.float32)
    for cc in range(NCH):
        for ocv in range(OC):
            onesA_all[ocv * OD:(ocv + 1) * OD, cc, cc * 8 + ocv] = 1.0
    c["onesA_all"] = onesA_all
    sel64 = np.zeros((64, 8), np.float32)        # sum over chunk blocks
    rep64 = np.zeros((8, 64), np.float32)        # replicate [8,1] -> [64,1]
    for cc in range(NCH):
        for ocv in range(OC):
            sel64[cc * 8 + ocv, ocv] = 1.0
            rep64[ocv, cc * 8 + ocv] = 1.0
    c["sel64"] = sel64
    c["sel64_16"] = sel64.astype(ml_dtypes.bfloat16)
    c["rep64"] = rep64
    # selrep[:, c, :]: [64, 8, 128] — replicate rows 8c..8c+7 (the chunk's
    # [8,CS] sigmoid block) over od into [128, CS]
    selrep = np.zeros((64, 8, 128), np.float32)
    for cc in range(NCH):
        for ocv in range(OC):
            selrep[cc * 8 + ocv, cc, ocv * OD:(ocv + 1) * OD] = 1.0
    c["selrep"] = selrep
    c["aw8"] = attn_w.reshape(OC, 1).astype(np.float32).copy()
    c["ab64"] = np.tile(attn_b.reshape(OC, 1), (8, 1)).astype(np.float32)
    c["zpad"] = np.zeros((128, 64), np.float32)
    return c


F32_CONSTS = {"b1s", "b2s", "b3s", "aw8", "ab64", "rep64"}
BF16_CONSTS = {"redM16", "onesB16", "i128_16", "ihalf_16", "sel64_16"}

CONST_SHAPES = {
    "w1L": [128, 128], "w2L": [128, 2, 9, 128], "w3L": [96, 8, 128],
    "b1s": [128, 2], "b2s": [128, 2], "b3s": [128, 8],
    "redM16": [128, 8, 64], "onesB16": [128, 64],
    "repM_both": [128, 8, 128],
    "i128_16": [128, 128], "ihalf_16": [128, 128],
    "onesA_all": [128, 8, 64], "sel64": [64, 8], "sel64_16": [64, 8],
    "rep64": [8, 64], "selrep": [64, 8, 128],
    "aw8": [8, 1], "ab64": [64, 1], "zpad": [128, 64],
}


def _cdt(name):
    return (F32 if name in F32_CONSTS else
            BF16 if name in BF16_CONSTS else F32R)


# ---------------------------------------------------------------- kernel IR
def build_nc(num_devices=8, stage=99):
    nc = bacc.Bacc("TRN2", target_bir_lowering=False, debug=False,
                   num_devices=num_devices)

    io = {}
    io["x"] = nc.dram_tensor("x", [128, PX], F32R, kind="ExternalInput").ap()
    for name, shp in CONST_SHAPES.items():
        io[name] = nc.dram_tensor(name, shp, _cdt(name),
                                  kind="ExternalInput").ap()
    out_dram = nc.dram_tensor("out", [128, PX], F32, kind="ExternalOutput").ap()

    with tile.TileContext(nc) as tc:
        _body(tc, io, out_dram, stage)
    nc.compile()
    return nc


def _body(tc, io, out_dram, stage=99):
    nc = tc.nc

    import contextlib
    ctx = contextlib.ExitStack()
    with ctx:
        consts = ctx.enter_context(tc.tile_pool(name="consts", bufs=1))
        cs_t = {}
        for name, shp in CONST_SHAPES.items():
            t = consts.tile(shp, _cdt(name), name=name, tag=name)
            nc.sync.dma_start(out=t[:], in_=io[name])
            cs_t[name] = t

        cb_eps = consts.tile([128, 1], F32, name="cb_eps", tag="cb_eps")
        nc.vector.memset(cb_eps[:], 1e-6)

        persist = ctx.enter_context(tc.tile_pool(name="persist", bufs=1))
        sf_sb = persist.tile([128, PX], F32R, name="sf", tag="sf")
        # flat h1 with 1-column zero margins (left/right) so conv2's dx taps
        # read full-width windows; the margins are zeroed once here.
        h1_sb = [persist.tile([128, H, W + 2], F32R, name=f"h1_{h}",
                              tag=f"h1_{h}") for h in range(2)]
        zp_col = cs_t["zpad"][:, 0:H].rearrange("p (a b) -> p a b", b=1)
        for h in range(2):
            nc.sync.dma_start(out=h1_sb[h][:, :, 0:1], in_=zp_col)
            nc.sync.dma_start(out=h1_sb[h][:, :, W + 1:W + 2], in_=zp_col)
        mh_parts = persist.tile([128, NCH], F32, name="mhp", tag="mhp")

        # main-loop pools (closed before the tail so tail pools reuse SBUF)
        ph2ps = contextlib.ExitStack()
        hb = ph2ps.enter_context(tc.tile_pool(name="hb", bufs=2))
        scr = ph2ps.enter_context(tc.tile_pool(name="scr", bufs=3))
        sm = ph2ps.enter_context(tc.tile_pool(name="sm", bufs=2))
        up = ph2ps.enter_context(tc.tile_pool(name="up", bufs=3))
        ppcv = ph2ps.enter_context(
            tc.tile_pool(name="ppcv", bufs=2, space="PSUM"))
        pppair = ph2ps.enter_context(
            tc.tile_pool(name="pppair", bufs=1, space="PSUM"))
        pps = ph2ps.enter_context(
            tc.tile_pool(name="pps", bufs=2, space="PSUM"))
        pprep = ph2ps.enter_context(
            tc.tile_pool(name="pprep", bufs=2, space="PSUM"))

        b3s = cs_t["b3s"]

        # -------------------------------------------------- conv pipeline
        def conv_front(c):
            """x DMA + relu + conv1 into the flat h1 tiles (chunk rows)."""
            xc = hb.tile([128, CS], F32R, name="xc", tag="xc")
            nc.sync.dma_start(out=xc[:], in_=io["x"][:, c * CS:(c + 1) * CS])
            rx = hb.tile([128, CS], F32R, name="rx", tag="rx")
            nc.scalar.activation(out=rx[:], in_=xc[:], func=AF.Relu)
            for h in range(2):
                ps = ppcv.tile([128, CS], F32, name="cvps", tag="cvps")
                nc.tensor.matmul(
                    ps[:], cs_t["w1L"][64 * h:64 * h + 64, :],
                    rx[64 * h:64 * h + 64, :], start=True, stop=True)
                nc.scalar.activation(
                    out=h1_sb[h][:, c * RPC:(c + 1) * RPC, 1:W + 1],
                    in_=ps[:].rearrange("p (a b) -> p a b", a=RPC),
                    func=AF.Relu, bias=cs_t["b1s"][:, h:h + 1], scale=1.0)

        def conv_back(c):
            """conv2 (tap-trimmed) + conv3 + u16 + sq16 + nsq reduce."""
            prow = 64 * (c % 2)
            h2b = [None, None]
            for h in range(2):
                ps = ppcv.tile([128, CS], F32, name="cvps", tag="cvps")
                for t, (dy, dx) in enumerate(TAPS):
                    sr0 = max(RPC * c + dy, 0)
                    sr1 = min(RPC * c + RPC + dy, H)
                    nr = sr1 - sr0
                    dr0 = sr0 - (RPC * c + dy)
                    nc.tensor.matmul(
                        ps[:, dr0 * W:(dr0 + nr) * W],
                        cs_t["w2L"][:, h, t, :],
                        h1_sb[h][:, sr0:sr1, 1 + dx:1 + dx + W],
                        start=(t == 0), stop=(t == len(TAPS) - 1),
                        skip_group_check=True)
                h2b[h] = hb.tile([128, CS], F32R, name=f"h2b{h}",
                                 tag=f"h2b{h}")
                nc.scalar.activation(
                    out=h2b[h][:], in_=ps[:],
                    func=AF.Relu, bias=cs_t["b2s"][:, h:h + 1], scale=1.0)

            u_c = []
            nsq_ps = _pair_ps(c, "nsq")
            for ic in range(IC):
                g = ic % 4
                if g < 3:
                    lhsT = cs_t["w3L"][32 * g:32 * g + 32, ic, :]
                    rhs = h2b[ic // 4][32 * g:32 * g + 32, :]
                else:
                    h2st = scr.tile([32, CS], F32R, name="h2s", tag="h2s")
                    nc.gpsimd.tensor_copy(out=h2st[:],
                                          in_=h2b[ic // 4][96:128, :])
                    lhsT = cs_t["w3L"][0:32, ic, :]
                    rhs = h2st[:]
                ups = ppcv.tile([128, CS], F32, name="cvps", tag="cvps")
                nc.tensor.matmul(ups[:], lhsT, rhs, start=True, stop=True)
                u_t = up.tile([128, CS], BF16, name=f"u{ic}", tag=f"u{ic}")
                nc.scalar.activation(out=u_t[:], in_=ups[:], func=AF.Identity,
                                     bias=b3s[:, ic:ic + 1], scale=1.0)
                sq_t = scr.tile([128, CS], BF16, name="sq", tag="sq")
                nc.scalar.activation(out=sq_t[:], in_=ups[:], func=AF.Square,
                                     bias=b3s[:, ic:ic + 1], scale=1.0)
                nc.tensor.matmul(nsq_ps[prow:prow + 64, :],
                                 cs_t["redM16"][:, ic, :], sq_t[:],
                                 start=(ic == 0), stop=(ic == IC - 1),
                                 skip_group_check=True)
                u_c.append(u_t)
            return u_c

        # pair-tile PSUM management: chunks 2P and 2P+1 share [128,CS] tiles
        _pair_tiles = {}

        def _pair_ps(c, kind):
            # "nsq" gets its own ring: it is allocated by conv_back of the
            # NEXT pair while the current pair's ns/d tiles are still in
            # flight — sharing a ring would create a cross-engine wait cycle.
            P = c // 2
            key = (P, kind)
            if key not in _pair_tiles:
                tag = "nsqp" if kind == "nsq" else "pair"
                _pair_tiles[key] = pppair.tile([128, CS], F32,
                                               name=kind, tag=tag)
            return _pair_tiles[key]

        def g_chain(nsq_ps, tag):
            """g = nsq/((0.5+nsq)*sqrt(nsq+1e-6)) on a [128,CS] pair tile.
            (drops the reference's final +1e-6 on the sqrt — relative error
            <= 1e-3 and well within tolerance)"""
            r_t = sm.tile([128, CS], F32, name="gr", tag="gr")
            nc.scalar.activation(out=r_t[:], in_=nsq_ps[:], func=AF.Sqrt,
                                 bias=cb_eps[:], scale=1.0)
            den = sm.tile([128, CS], F32, name="gd", tag="gd")
            nc.vector.scalar_tensor_tensor(out=den[:], in0=nsq_ps[:],
                                           scalar=0.5, in1=r_t[:],
                                           op0=OP.add, op1=OP.mult)
            rg = sm.tile([128, CS], F32, name="grg", tag="grg")
            nc.vector.reciprocal_approx_fast(out=rg[:], in_=den[:])
            g_t = sm.tile([128, CS], F32R, name=tag, tag=tag)
            nc.vector.tensor_tensor(out=g_t[:], in0=nsq_ps[:], in1=rg[:],
                                    op=OP.mult)
            return g_t

        def accum_pass(c, u_c, coef_t, out_ps, accI, keep=None):
            """out_ps = sum_ic u16[ic] * (repM[ic] @ coef_t) for one chunk.
            The TT products optionally land in `keep` (the usq tiles)."""
            prow = 64 * (c % 2)
            for ic in range(IC):
                rep = pprep.tile([128, CS], F32, name="rep", tag="rep")
                nc.tensor.matmul(rep[:], cs_t["repM_both"][prow:prow + 64,
                                                           ic, :],
                                 coef_t[prow:prow + 64, :],
                                 start=True, stop=True)
                if keep is not None:
                    p_t = keep[ic]
                else:
                    p_t = scr.tile([128, CS], BF16, name="p16", tag="p16")
                nc.vector.tensor_tensor(out=p_t[:], in0=u_c[ic][:],
                                        in1=rep[:], op=OP.mult)
                nc.tensor.matmul(out_ps[:], accI[:], p_t[:],
                                 start=(ic == 0), stop=(ic == IC - 1))

        def s_stats(c, s_ps, ns_ps):
            """s16 (bf16 copy of s psum) + ns = PE reduce of Square(s)."""
            prow = 64 * (c % 2)
            s16 = sm.tile([128, CS], BF16, name=f"s16_{c % 2}",
                          tag=f"s16_{c % 2}")
            nc.scalar.activation(out=s16[:], in_=s_ps[:], func=AF.Identity)
            ssq = scr.tile([128, CS], BF16, name="ssq", tag="ssq")
            nc.scalar.activation(out=ssq[:], in_=s_ps[:], func=AF.Square)
            nc.tensor.matmul(ns_ps[prow:prow + 64, :], cs_t["onesB16"][:],
                             ssq[:], start=True, stop=True,
                             skip_group_check=True)
            return s16

        def dred_pass(c, u_c, s16, out_ps):
            """out_ps rows (pair half) += per-ic od-reduction of u*s16.
            Most of the TT multiplies go to the otherwise-idle Pool engine."""
            prow = 64 * (c % 2)
            for ic in range(IC):
                q_t = scr.tile([128, CS], BF16, name="q16", tag="q16")
                eng = nc.vector if ic < 2 else nc.gpsimd
                eng.tensor_tensor(out=q_t[:], in0=u_c[ic][:],
                                  in1=s16[:], op=OP.mult)
                nc.tensor.matmul(out_ps[prow:prow + 64, :],
                                 cs_t["redM16"][:, ic, :], q_t[:],
                                 start=(ic == 0), stop=(ic == IC - 1),
                                 skip_group_check=True)

        def routing_pair(P, u_AB, fills):
            """Full routing for chunk pair (2P, 2P+1). `fills` is a list of
            callbacks emitting next-pair conv work, interleaved at fixed
            points for cross-engine overlap."""
            A, Bc = 2 * P, 2 * P + 1

            def fill(i):
                if i < len(fills):
                    fills[i]()

            nsq_ps = _pair_ps(A, "nsq")
            g_u = g_chain(nsq_ps, "g_u")

            # iter 1: uniform c=0.5 folded into the 0.5*I accumulate; the TT
            # products usq = g*u are kept for reuse in iter 2's mixes
            usq = [[up.tile([128, CS], BF16, name=f"usq{i}_{ic}",
                            tag=f"usq{i}_{ic}", bufs=1) for ic in range(IC)]
                   for i in range(2)]
            s1_ps = [pps.tile([128, CS], F32, name="sacc", tag="sacc")
                     for _ in range(2)]
            accum_pass(A, u_AB[0], g_u, s1_ps[0], cs_t["ihalf_16"],
                       keep=usq[0])
            fill(0)
            accum_pass(Bc, u_AB[1], g_u, s1_ps[1], cs_t["ihalf_16"],
                       keep=usq[1])

            ns1_ps = _pair_ps(A, "ns1")
            s16A = s_stats(A, s1_ps[0], ns1_ps)
            s16B = s_stats(Bc, s1_ps[1], ns1_ps)
            fill(1)
            g1 = g_chain(ns1_ps, "g1")

            # d1' = sum_od usq*s1 = g*d1, so b2 = g1*d1' directly
            d1_ps = _pair_ps(A, "d1")
            dred_pass(A, usq[0], s16A, d1_ps)
            dred_pass(Bc, usq[1], s16B, d1_ps)

            b2 = sm.tile([128, CS], F32R, name="b2", tag="b2")
            nc.vector.tensor_tensor(out=b2[:], in0=d1_ps[:], in1=g1[:],
                                    op=OP.mult)
            c2 = sm.tile([128, CS], F32R, name="c2", tag="c2")
            nc.scalar.activation(out=c2[:], in_=b2[:], func=AF.Sigmoid)
            fill(2)

            # iter 2: s2 = sum_ic rep(c2) * usq
            s2_ps = [pps.tile([128, CS], F32, name="sacc", tag="sacc")
                     for _ in range(2)]
            accum_pass(A, usq[0], c2, s2_ps[0], cs_t["i128_16"])
            accum_pass(Bc, usq[1], c2, s2_ps[1], cs_t["i128_16"])

            ns2_ps = _pair_ps(A, "ns2")
            s16A2 = s_stats(A, s2_ps[0], ns2_ps)
            s16B2 = s_stats(Bc, s2_ps[1], ns2_ps)
            fill(3)
            g2 = g_chain(ns2_ps, "g2")

            d2_ps = _pair_ps(A, "d2")
            dred_pass(A, usq[0], s16A2, d2_ps)
            dred_pass(Bc, usq[1], s16B2, d2_ps)

            t9 = sm.tile([128, CS], F32R, name="t9", tag="t9")
            nc.vector.tensor_tensor(out=t9[:], in0=d2_ps[:], in1=g2[:],
                                    op=OP.mult)
            b3r = sm.tile([128, CS], F32R, name="b3r", tag="b3r")
            nc.gpsimd.tensor_tensor(out=b3r[:], in0=t9[:], in1=b2[:],
                                    op=OP.add)
            c3 = sm.tile([128, CS], F32R, name="c2", tag="c2")
            nc.scalar.activation(out=c3[:], in_=b3r[:], func=AF.Sigmoid)
            fill(4)

            # final mix sf = sum_ic rep(c3)*u + sf copies (mh partial sums
            # ride along via the activation accumulator)
            for idx, c in enumerate((A, Bc)):
                sf_ps = pps.tile([128, CS], F32, name="sacc", tag="sacc")
                accum_pass(c, u_AB[idx], c3, sf_ps, cs_t["i128_16"])
                nc.scalar.activation(
                    out=sf_sb[:, c * CS:(c + 1) * CS], in_=sf_ps[:],
                    func=AF.Identity,
                    accum_out=mh_parts[:, c:c + 1])
            for i in range(5, len(fills)):
                fills[i]()

        # ---------------------------- emit conv prologue + routing loop
        conv_front(0)
        conv_front(1)
        u_first = conv_back(0)
        conv_front(2)
        u_AB = [u_first, conv_back(1)]

        u_next = [None, None]

        for P in range(4):
            fills = []
            if P < 3:
                cA, cB = 2 * P + 2, 2 * P + 3

                def mk(fn, *args, store=None, idx=None):
                    def run():
                        r = fn(*args)
                        if store is not None:
                            store[idx] = r
                    return run
                fills = [
                    mk(conv_front, cA + 1),
                    mk(conv_back, cA, store=u_next, idx=0),
                    mk(conv_front, cB + 1) if cB + 1 < NCH else (lambda: None),
                    mk(conv_back, cB, store=u_next, idx=1),
                ]
            routing_pair(P, u_AB, fills)
            if P < 3:
                u_AB = [u_next[0], u_next[1]]
                u_next = [None, None]

        if stage <= 4:
            ph2ps.close()
            nc.sync.dma_start(out=out_dram, in_=sf_sb[:])
            return

        # ---------------- tail: spatial capsule attention ----------------
        ph2ps.close()
        tt = ctx.enter_context(tc.tile_pool(name="tt", bufs=1))
        tl = ctx.enter_context(tc.tile_pool(name="tl", bufs=2))
        ppt = ctx.enter_context(tc.tile_pool(name="ppt", bufs=1, space="PSUM"))
        ppt2 = ctx.enter_context(
            tc.tile_pool(name="ppt2", bufs=2, space="PSUM"))

        mh = tt.tile([128, 1], F32, name="mh", tag="mh")
        nc.vector.reduce_sum(out=mh[:], in_=mh_parts[:],
                             axis=mybir.AxisListType.X)
        nc.scalar.mul(mh[:], mh[:], 1.0 / PX)

        # avg64 accumulated in PSUM: rows 8c+oc = sum_od sf*mh of chunk c
        avg_ps = ppt.tile([64, CS], F32, name="avgps", tag="avgps")
        for c in range(NCH):
            csl = slice(c * CS, (c + 1) * CS)
            scrc = tl.tile([128, CS], F32R, name="scrc", tag="scrc")
            nc.scalar.activation(out=scrc[:], in_=sf_sb[:, csl],
                                 func=AF.Identity, scale=mh[:])
            nc.tensor.matmul(avg_ps[:], cs_t["onesA_all"][:, c, :], scrc[:],
                             start=(c == 0), stop=(c == NCH - 1),
                             skip_group_check=True)
        avg64 = tt.tile([64, CS], F32R, name="avg64", tag="avg64")
        nc.scalar.activation(out=avg64[:], in_=avg_ps[:], func=AF.Identity)

        # mean over (chunk, px) per oc, broadcast back to [64,1]
        m_ps = ppt2.tile([8, CS], F32, name="m8", tag="t8")
        nc.tensor.matmul(m_ps[:], cs_t["sel64"][:], avg64[:],
                         start=True, stop=True)
        mrow = tt.tile([8, 1], F32, name="mrow", tag="mrow")
        nc.vector.reduce_sum(out=mrow[:], in_=m_ps[:],
                             axis=mybir.AxisListType.X)
        m64_ps = ppt2.tile([64, 1], F32, name="m64ps", tag="t64")
        nc.tensor.matmul(m64_ps[:], cs_t["rep64"][:], mrow[:],
                         start=True, stop=True)
        m64 = tt.tile([64, 1], F32, name="m64", tag="m64")
        nc.scalar.activation(out=m64[:], in_=m64_ps[:], func=AF.Identity,
                             scale=1.0 / PX)
        cen = tt.tile([64, CS], F32R, name="cen", tag="cen")
        nc.vector.tensor_scalar(out=cen[:], in0=avg64[:], scalar1=m64[:],
                                scalar2=None, op0=OP.subtract)
        vj16 = tt.tile([64, CS], BF16, name="vj16", tag="vj16")
        nc.vector.tensor_tensor(out=vj16[:], in0=cen[:], in1=cen[:],
                                op=OP.mult)
        v_ps = ppt2.tile([8, CS], F32, name="v8", tag="t8")
        nc.tensor.matmul(v_ps[:], cs_t["sel64_16"][:], vj16[:],
                         start=True, stop=True)
        vrow = tt.tile([8, 1], F32, name="vrow", tag="mrow")
        nc.vector.reduce_sum(out=vrow[:], in_=v_ps[:],
                             axis=mybir.AxisListType.X)
        sd8 = tt.tile([8, 1], F32, name="sd8", tag="sd8")
        nc.scalar.activation(out=sd8[:], in_=vrow[:], func=AF.Sqrt,
                             bias=0.0, scale=1.0 / (PX - 1))
        nc.scalar.activation(out=sd8[:], in_=sd8[:], func=AF.Identity,
                             bias=cb_eps[:8], scale=1.0)
        rsd = tt.tile([8, 1], F32, name="rsd", tag="rsd")
        nc.vector.reciprocal(out=rsd[:], in_=sd8[:])
        rsdw = tt.tile([8, 1], F32, name="rsdw", tag="rsdw")
        nc.vector.tensor_tensor(out=rsdw[:], in0=rsd[:], in1=cs_t["aw8"][:],
                                op=OP.mult)
        rw64_ps = ppt2.tile([64, 1], F32, name="rw64ps", tag="t64")
        nc.tensor.matmul(rw64_ps[:], cs_t["rep64"][:], rsdw[:],
                         start=True, stop=True)
        rw64 = tt.tile([64, 1], F32, name="rw64", tag="rw64")
        nc.scalar.activation(out=rw64[:], in_=rw64_ps[:], func=AF.Identity)
        t2 = tt.tile([64, CS], F32, name="t2", tag="t2")
        nc.vector.tensor_scalar(out=t2[:], in0=cen[:], scalar1=rw64[:],
                                scalar2=cs_t["ab64"][:], op0=OP.mult,
                                op1=OP.add)
        sig = tt.tile([64, CS], F32R, name="sig", tag="sig")
        nc.scalar.activation(out=sig[:], in_=t2[:], func=AF.Sigmoid)

        # final: out = sf * sigmoid-rep + x, split across DVE and Pool
        ppsr = ctx.enter_context(
            tc.tile_pool(name="ppsr", bufs=2, space="PSUM"))
        for c in range(NCH):
            csl = slice(c * CS, (c + 1) * CS)
            srep = ppsr.tile([128, CS], F32, name="srep", tag="srep")
            nc.tensor.matmul(srep[:], cs_t["selrep"][:, c, :], sig[:],
                             start=True, stop=True)
            xr = tl.tile([128, CS], F32R, name="xr", tag="xr")
            nc.sync.dma_start(out=xr[:], in_=io["x"][:, csl])
            o1 = tl.tile([128, CS], F32, name="o1", tag="o1")
            o2 = tl.tile([128, CS], F32, name="o2", tag="o2")
            # o1 reads PSUM so it must stay off GPSIMD; o2 is SBUF-only
            nc.vector.tensor_tensor(out=o1[:], in0=sf_sb[:, csl],
                                    in1=srep[:], op=OP.mult)
            eng = nc.vector if c % 2 == 0 else nc.gpsimd
            eng.tensor_tensor(out=o2[:], in0=o1[:], in1=xr[:], op=OP.add)
            nc.sync.dma_start(out=out_dram[:, csl], in_=o2[:])


# ---------------------------------------------------------------- dispatch
_NC_CACHE = {}


def _get_nc():
    if "nc" not in _NC_CACHE:
        _NC_CACHE["nc"] = build_nc()
    return _NC_CACHE["nc"]


def kernel(x, w1, b1, w2, b2, w3, b3, attn_w, attn_b):
    x = np.ascontiguousarray(np.asarray(x, dtype=np.float32))
    consts = _prep_consts(
        np.asarray(w1, np.float32), np.asarray(b1, np.float32),
        np.asarray(w2, np.float32), np.asarray(b2, np.float32),
        np.asarray(w3, np.float32), np.asarray(b3, np.float32),
        np.asarray(attn_w, np.float32), np.asarray(attn_b, np.float32))
    consts = {k: np.ascontiguousarray(v) for k, v in consts.items()}

    nc = _get_nc()
    in_maps = []
    for b in range(B):
        m = {"x": x[b].reshape(128, PX).copy()}
        m.update(consts)
        in_maps.append(m)
    res = run_bass_kernel_spmd(nc, in_maps, core_ids=list(range(B)))
    out = np.zeros((B, 128, H, W), np.float32)
    for b in range(B):
        out[b] = res.results[b]["out"].reshape(128, H, W)
    return out
